# revision 1
# baseline (speedup 1.0000x reference)
"""Trainium2 Bass kernel for nn_LocallyDense (grouped gather + per-group Dense
+ LeakyReLU + BatchNorm inference).

Sharding: expert-parallel over the 41 groups across 8 cores (6 groups/core,
padded with a duplicate group on 5-group cores so one SPMD program fits all).
Each core receives:
  - xt:   the (deduplicated) set of x columns its groups reference, as rows
          [UPAD, B] (x transposed so each needed column is a contiguous row)
  - wt:   its groups' weights [NG*G, D_OUT], rows permuted to match the
          sorted gather order (sum over k is permutation invariant)
  - idxw: int16 gather indices in the SWDGE wrapped layout [128, NG*G/16]
  - bias: [NG, D_OUT], bn: [4, D_OUT] (gamma, beta, moving_mean, moving_var)
On device: dma_gather builds lhsT K-tiles [128, KT, B]; TensorE accumulates
out[b, o] per group in PSUM (bias folded in as a K=1 matmul row); epilogue is
leaky = max(psum, alpha*psum) (one fused DVE op) then BN affine y=t*inv+c with
inv/c computed on device and partition-broadcast.
"""

import numpy as np
import ml_dtypes

B, D_IN, N_GROUPS, G, D_OUT = 256, 65536, 41, 1536, 256
BN_EPS = 1e-3
ALPHA = 0.3
N_CORES = 8
NG = 6                # groups per core (padded)
KT = G // 128         # 12 K-tiles per group
UPAD = NG * G         # padded unique-column table rows (9216)
IDX_COLS = NG * G // 16

USE_BF16 = True       # x/W feed the PE in bf16 (fp32 accumulate in PSUM)
TRACE = False         # set by test.py for profiling runs
TRACE_KW = {}
REPEAT = 1            # run the main loop R times (benchmarking differential)

_prog_cache = {}


def _np_dtx():
    return ml_dtypes.bfloat16 if USE_BF16 else np.float32


def _build_program(use_bf16: bool):
    import concourse.bacc as bacc
    import concourse.mybir as mybir
    import concourse.tile as tile
    from concourse.library_config import mlp as mlp_lib

    f32 = mybir.dt.float32
    dt_x = mybir.dt.bfloat16 if use_bf16 else mybir.dt.float32

    nc = bacc.Bacc("TRN2", target_bir_lowering=False, debug=False,
                   num_devices=N_CORES)
    xt = nc.dram_tensor("xt", [UPAD, B], dt_x, kind="ExternalInput")
    wt = nc.dram_tensor("wt", [NG * G, D_OUT], dt_x, kind="ExternalInput")
    idxw = nc.dram_tensor("idxw", [128, IDX_COLS], mybir.dt.int16,
                          kind="ExternalInput")
    bias = nc.dram_tensor("bias", [NG, D_OUT], f32, kind="ExternalInput")
    bn = nc.dram_tensor("bn", [4, D_OUT], f32, kind="ExternalInput")
    out = nc.dram_tensor("out", [B, NG * D_OUT], f32, kind="ExternalOutput")

    with tile.TileContext(nc) as tc:
        with tc.tile_pool(name="const", bufs=1) as cpool, \
             tc.tile_pool(name="gat", bufs=2) as gpool, \
             tc.tile_pool(name="wp", bufs=6) as wpool, \
             tc.tile_pool(name="ep", bufs=4) as epool, \
             tc.tile_pool(name="ps", bufs=3, space="PSUM") as ppool:

            nc.gpsimd.load_library(mlp_lib)

            idx_t = cpool.tile([128, IDX_COLS], mybir.dt.int16)
            nc.sync.dma_start(out=idx_t[:], in_=idxw[:, :])

            bn_rows = []
            for r in range(4):
                bt = cpool.tile([1, D_OUT], f32, name=f"bn_{r}")
                nc.sync.dma_start(out=bt[:], in_=bn[r:r + 1, :])
                bn_rows.append(bt)

            bias_ts = []
            for g in range(NG):
                bt = cpool.tile([1, D_OUT], f32, tag=f"bias{g}")
                nc.sync.dma_start(out=bt[:], in_=bias[g:g + 1, :])
                bias_ts.append(bt)

            ones1 = cpool.tile([1, 128], f32)
            nc.vector.memset(ones1[:], 1.0)

            # BN: inv = gamma / sqrt(var + eps);  c = beta - mean * inv
            inv1 = cpool.tile([1, D_OUT], f32)
            c1 = cpool.tile([1, D_OUT], f32)
            tmp1 = cpool.tile([1, D_OUT], f32)
            nc.vector.tensor_scalar_add(tmp1[:], bn_rows[3][:], BN_EPS)
            nc.scalar.sqrt(tmp1[:], tmp1[:])
            nc.vector.reciprocal(tmp1[:], tmp1[:])
            nc.vector.tensor_mul(inv1[:], tmp1[:], bn_rows[0][:])
            nc.vector.tensor_mul(tmp1[:], bn_rows[2][:], inv1[:])
            nc.vector.tensor_sub(c1[:], bn_rows[1][:], tmp1[:])
            invB = cpool.tile([128, D_OUT], f32)
            cB = cpool.tile([128, D_OUT], f32)
            # broadcast [1,256] -> [128,256] via ones[1,128]^T @ v[1,256]
            for src, dst, nm in ((inv1, invB, "binv"), (c1, cB, "bc")):
                bps = ppool.tile([128, D_OUT], f32, tag="ps0", name=f"bps_{nm}")
                nc.tensor.matmul(out=bps[:], lhsT=ones1[:], rhs=src[:],
                                 start=True, stop=True)
                nc.vector.tensor_copy(dst[:], bps[:])

            for g_rep in range(REPEAT * NG):
                g = g_rep % NG
                gat = gpool.tile([128, KT, B], dt_x, tag="gat")
                nc.gpsimd.dma_gather(
                    gat[:], xt[:, :], idx_t[:, g * (G // 16):(g + 1) * (G // 16)],
                    G, G, B, single_packet=False)
                psums = [ppool.tile([128, D_OUT], f32, tag=f"ps{h}",
                                    name=f"ps{h}_{g_rep}")
                         for h in range(2)]
                for h in range(2):
                    nc.tensor.matmul(out=psums[h][:], lhsT=ones1[:],
                                     rhs=bias_ts[g][:], start=True, stop=False)
                for blk in range(KT):
                    wtile = wpool.tile([128, D_OUT], dt_x, tag="w")
                    nc.sync.dma_start(
                        out=wtile[:],
                        in_=wt[g * G + blk * 128: g * G + (blk + 1) * 128, :])
                    for h in range(2):
                        nc.tensor.matmul(out=psums[h][:],
                                         lhsT=gat[:, blk, h * 128:(h + 1) * 128],
                                         rhs=wtile[:],
                                         start=False, stop=(blk == KT - 1))
                for h in range(2):
                    ot = epool.tile([128, D_OUT], f32, tag="ot")
                    rt = epool.tile([128, D_OUT], f32, tag="rt")
                    # leaky(x) = alpha*x + (1-alpha)*relu(x); ACT does the
                    # scaled relu (one PSUM read), DVE fuses the rest
                    nc.scalar.activation(out=rt[:], in_=psums[h][:],
                                         func=mybir.ActivationFunctionType.Relu,
                                         scale=float(1.0 - ALPHA))
                    nc.vector.scalar_tensor_tensor(
                        out=ot[:], in0=psums[h][:], scalar=ALPHA,
                        in1=rt[:],
                        op0=mybir.AluOpType.mult, op1=mybir.AluOpType.add)
                    nc.vector.tensor_mul(ot[:], ot[:], invB[:])
                    nc.vector.tensor_add(ot[:], ot[:], cB[:])
                    nc.sync.dma_start(
                        out=out[h * 128:(h + 1) * 128,
                                g * D_OUT:(g + 1) * D_OUT],
                        in_=ot[:])
    nc.compile()
    return nc


def _get_program(use_bf16: bool):
    key = (use_bf16, REPEAT)
    if key not in _prog_cache:
        _prog_cache[key] = _build_program(use_bf16)
    return _prog_cache[key]


def _prep_inputs(x, gidx, W, b, gamma, beta, mmean, mvar):
    dtx = _np_dtx()
    xT = np.ascontiguousarray(x.T)  # [D_IN, B]
    assign = [list(range(0, 6))] + \
             [list(range(6 + 5 * i, 6 + 5 * (i + 1))) for i in range(7)]
    bn_arr = np.ascontiguousarray(
        np.stack([gamma, beta, mmean, mvar]).astype(np.float32))
    in_maps, metas = [], []
    for c in range(N_CORES):
        gs = assign[c]
        real = len(gs)
        gs6 = gs + [gs[-1]] * (NG - real)
        gi = gidx[gs6]  # [NG, G]
        uniq, inv = np.unique(gi, return_inverse=True)
        inv = inv.reshape(NG, G)
        xtc = np.zeros((UPAD, B), dtype=dtx)
        xtc[:len(uniq)] = xT[uniq].astype(dtx)
        Wc = np.empty((NG * G, D_OUT), dtype=dtx)
        idx16 = np.empty((NG, G), np.int16)
        for j in range(NG):
            order = np.argsort(inv[j], kind="stable")
            idx16[j] = inv[j][order].astype(np.int16)
            Wc[j * G:(j + 1) * G] = W[gs6[j]][order].astype(dtx)
        # SWDGE wrapped layout: idx i -> partition i%16, column i//16,
        # replicated across the 8 Q7 cores (16-partition stripes x 8)
        wr = idx16.reshape(NG, G // 16, 16).transpose(0, 2, 1)  # [j, p, s]
        wr = np.concatenate(list(wr), axis=1)  # [16, IDX_COLS]
        idxw_arr = np.ascontiguousarray(np.tile(wr, (8, 1)))  # [128, IDX_COLS]
        bc = np.ascontiguousarray(b[gs6].astype(np.float32))
        in_maps.append({"xt": xtc, "wt": Wc, "idxw": idxw_arr,
                        "bias": bc, "bn": bn_arr})
        metas.append((gs, real))
    return in_maps, metas


def kernel(**inputs):
    x = np.asarray(inputs["x"], dtype=np.float32)
    gidx = np.asarray(inputs["group_idx"]).astype(np.int64)
    W = np.asarray(inputs["W"], dtype=np.float32)
    b = np.asarray(inputs["b"], dtype=np.float32)
    gamma = np.asarray(inputs["gamma"], dtype=np.float32)
    beta = np.asarray(inputs["beta"], dtype=np.float32)
    mmean = np.asarray(inputs["moving_mean"], dtype=np.float32)
    mvar = np.asarray(inputs["moving_var"], dtype=np.float32)

    in_maps, metas = _prep_inputs(x, gidx, W, b, gamma, beta, mmean, mvar)
    nc = _get_program(USE_BF16)

    from concourse import bass_utils
    res = bass_utils.run_bass_kernel_spmd(
        nc, in_maps, core_ids=list(range(N_CORES)), trace=TRACE, **TRACE_KW)
    if TRACE:
        kernel.last_result = res

    full = np.empty((B, N_GROUPS, D_OUT), dtype=np.float32)
    for c, (gs, real) in enumerate(metas):
        o = res.results[c]["out"].reshape(B, NG, D_OUT)
        full[:, gs, :] = o[:, :real, :]
    return full


def run_sim(core=0):
    """CoreSim validation of one core's program (no hardware)."""
    import sys
    sys.path.insert(0, "/root/problem")
    from test import load_ref
    from concourse.bass_interp import CoreSim
    inputs, expected = load_ref()
    x = inputs["x"].astype(np.float32)
    gidx = inputs["group_idx"].astype(np.int64)
    in_maps, metas = _prep_inputs(
        x, gidx, inputs["W"].astype(np.float32), inputs["b"].astype(np.float32),
        inputs["gamma"].astype(np.float32), inputs["beta"].astype(np.float32),
        inputs["moving_mean"].astype(np.float32),
        inputs["moving_var"].astype(np.float32))
    nc = _get_program(USE_BF16)
    sim = CoreSim(nc)
    sim.assign_tensors(in_maps[core])
    sim.simulate(check_with_hw=False)
    o = sim.tensor("out").reshape(B, NG, D_OUT)
    gs, real = metas[core]
    exp_c = expected[:, gs, :]
    act_c = o[:, :real, :]
    err = np.max(np.abs(act_c - exp_c)) / (np.max(np.abs(exp_c)) + 1e-30)
    print(f"core {core}: sim max-abs-rel err = {err:.3e}")
    return err


if __name__ == "__main__":
    run_sim(0)



# revision 4
# speedup vs baseline: 2.5243x; 2.5243x over previous
"""Trainium2 Bass kernel for nn_LocallyDense (grouped gather + per-group Dense
+ LeakyReLU + BatchNorm inference).

Sharding: expert-parallel over the 41 groups across 8 cores (6 groups on
core 0, 5 on cores 1-7, padded to 6 with a duplicate so one SPMD program
fits all).

The gather (x columns per group) and all BN constant math happen on the
HOST during input prep — the device program is a pure streamed GEMM in the
transposed formulation out^T[o, b] = W^T x^T:
  - lhsT (stationary) = W K-tile  [K=128, M=128 output-half]
  - rhs  (moving)     = gathered-x K-tile [K=128, N=256 batch]
  - PSUM accumulates 12 K-tiles per (group, output-half)
Epilogue is 2 ACT ops per tile, using per-partition scalar APs (the
transposed layout puts the output-feature dim on partitions):
  t = Prelu(psum + bias, alpha=0.3); y = Identity(t*inv + c)
with inv = gamma/sqrt(var+eps), c = beta - mean*inv precomputed on host.
The host transposes the returned out^T tiles back while unsharding.
"""

import numpy as np
import ml_dtypes

B, D_IN, N_GROUPS, G, D_OUT = 256, 65536, 41, 1536, 256
BN_EPS = 1e-3
ALPHA = 0.3
N_CORES = 8
NG = 6                # groups per core (padded)
KT = G // 128         # 12 K-tiles per group

USE_BF16 = True       # x/W feed the PE in bf16 (fp32 accumulate in PSUM)
TRACE = False         # set by test.py for profiling runs
TRACE_KW = {}
REPEAT = 1            # run the main loop R times (benchmarking differential)

_prog_cache = {}


def _np_dtx():
    return ml_dtypes.bfloat16 if USE_BF16 else np.float32


def _build_program(use_bf16: bool):
    import concourse.bacc as bacc
    import concourse.mybir as mybir
    import concourse.tile as tile

    f32 = mybir.dt.float32
    dt_x = mybir.dt.bfloat16 if use_bf16 else mybir.dt.float32

    nc = bacc.Bacc("TRN2", target_bir_lowering=False, debug=False,
                   num_devices=N_CORES)
    xg = nc.dram_tensor("xg", [128, NG * KT * B], dt_x, kind="ExternalInput")
    wt = nc.dram_tensor("wt", [128, NG * KT * D_OUT], dt_x,
                        kind="ExternalInput")
    # cols 0-11: bias[g, h*128+p]; 12-13: inv[h*128+p]; 14-15: c[h*128+p]
    cons = nc.dram_tensor("cons", [128, 16], f32, kind="ExternalInput")
    out = nc.dram_tensor("out", [NG * 2 * 128, B], f32, kind="ExternalOutput")

    with tile.TileContext(nc) as tc:
        with tc.tile_pool(name="const", bufs=1) as cpool, \
             tc.tile_pool(name="xp", bufs=NG) as xpool, \
             tc.tile_pool(name="wp", bufs=NG) as wpool, \
             tc.tile_pool(name="ep", bufs=4) as epool, \
             tc.tile_pool(name="ps", bufs=3, space="PSUM") as ppool:

            ct = cpool.tile([128, 16], f32)
            nc.sync.dma_start(out=ct[:], in_=cons[:, :])

            for g_rep in range(REPEAT * NG):
                g = g_rep % NG
                xt = xpool.tile([128, KT, B], dt_x, tag="x")
                nc.sync.dma_start(out=xt[:], in_=xg[:, g * KT * B:
                                                   (g + 1) * KT * B])
                wtt = wpool.tile([128, KT, D_OUT], dt_x, tag="w")
                nc.sync.dma_start(out=wtt[:], in_=wt[:, g * KT * D_OUT:
                                                     (g + 1) * KT * D_OUT])
                for h in range(2):
                    ps = ppool.tile([128, B], f32, tag=f"ps{h}",
                                    name=f"ps{h}_{g_rep}")
                    for blk in range(KT):
                        nc.tensor.matmul(
                            out=ps[:],
                            lhsT=wtt[:, blk, h * 128:(h + 1) * 128],
                            rhs=xt[:, blk, :],
                            start=(blk == 0), stop=(blk == KT - 1))
                    t = epool.tile([128, B], f32, tag="t")
                    nc.scalar.activation(
                        out=t[:], in_=ps[:],
                        func=mybir.ActivationFunctionType.Prelu,
                        bias=ct[:, 2 * g + h:2 * g + h + 1],
                        scale=1.0, alpha=float(ALPHA))
                    y = epool.tile([128, B], f32, tag="y")
                    nc.vector.tensor_scalar(
                        out=y[:], in0=t[:],
                        scalar1=ct[:, 12 + h:13 + h],
                        scalar2=ct[:, 14 + h:15 + h],
                        op0=mybir.AluOpType.mult,
                        op1=mybir.AluOpType.add)
                    nc.scalar.dma_start(
                        out=out[(g * 2 + h) * 128:(g * 2 + h + 1) * 128, :],
                        in_=y[:])
    nc.compile()
    return nc


def _get_program(use_bf16: bool):
    key = (use_bf16, REPEAT)
    if key not in _prog_cache:
        _prog_cache[key] = _build_program(use_bf16)
    return _prog_cache[key]


def _prep_inputs(x, gidx, W, b, gamma, beta, mmean, mvar):
    dtx = _np_dtx()
    assign = [list(range(0, 6))] + \
             [list(range(6 + 5 * i, 6 + 5 * (i + 1))) for i in range(7)]
    inv = (gamma.astype(np.float64) /
           np.sqrt(mvar.astype(np.float64) + BN_EPS)).astype(np.float32)
    cvec = (beta - mmean * inv).astype(np.float32)
    inv_pc = inv.reshape(2, 128).T      # [128, 2]
    c_pc = cvec.reshape(2, 128).T       # [128, 2]
    in_maps, metas = [], []
    for c in range(N_CORES):
        gs = assign[c]
        real = len(gs)
        gs6 = gs + [gs[-1]] * (NG - real)
        gi = gidx[gs6]                                   # [NG, G]
        cols = gi.reshape(-1)                            # [NG*G]
        A = x[:, cols]                                   # [B, NG*G] gather
        xg_sb = np.ascontiguousarray(
            A.T.reshape(NG, KT, 128, B).transpose(2, 0, 1, 3)
        ).astype(dtx).reshape(128, NG * KT * B)
        wt_sb = np.ascontiguousarray(
            W[gs6].reshape(NG, KT, 128, D_OUT).transpose(2, 0, 1, 3)
        ).astype(dtx).reshape(128, NG * KT * D_OUT)
        cons = np.zeros((128, 16), np.float32)
        cons[:, 0:12] = b[gs6].reshape(NG, 2, 128).transpose(2, 0, 1) \
                              .reshape(128, 12)
        cons[:, 12:14] = inv_pc
        cons[:, 14:16] = c_pc
        in_maps.append({"xg": xg_sb, "wt": wt_sb,
                        "cons": np.ascontiguousarray(cons)})
        metas.append((gs, real))
    return in_maps, metas


def kernel(**inputs):
    x = np.asarray(inputs["x"], dtype=np.float32)
    gidx = np.asarray(inputs["group_idx"]).astype(np.int64)
    W = np.asarray(inputs["W"], dtype=np.float32)
    b = np.asarray(inputs["b"], dtype=np.float32)
    gamma = np.asarray(inputs["gamma"], dtype=np.float32)
    beta = np.asarray(inputs["beta"], dtype=np.float32)
    mmean = np.asarray(inputs["moving_mean"], dtype=np.float32)
    mvar = np.asarray(inputs["moving_var"], dtype=np.float32)

    in_maps, metas = _prep_inputs(x, gidx, W, b, gamma, beta, mmean, mvar)
    nc = _get_program(USE_BF16)

    from concourse import bass_utils
    res = bass_utils.run_bass_kernel_spmd(
        nc, in_maps, core_ids=list(range(N_CORES)), trace=TRACE, **TRACE_KW)
    if TRACE:
        kernel.last_result = res

    full = np.empty((B, N_GROUPS, D_OUT), dtype=np.float32)
    for c, (gs, real) in enumerate(metas):
        o = res.results[c]["out"].reshape(NG, 2, 128, B)   # [g, h, p, b]
        oc = o.transpose(3, 0, 1, 2).reshape(B, NG, D_OUT)  # [b, g, o]
        full[:, gs, :] = oc[:, :real, :]
    return full


def host_check():
    """Validate host prep + unshard logic with a numpy matmul (no device)."""
    d = np.load("/root/problem/_ref_cache.npz")
    x = d["x"].astype(np.float32)
    gidx = d["group_idx"].astype(np.int64)
    W, b = d["W"].astype(np.float32), d["b"].astype(np.float32)
    expected = d["expected"]
    in_maps, metas = _prep_inputs(
        x, gidx, W, b, d["gamma"].astype(np.float32),
        d["beta"].astype(np.float32), d["moving_mean"].astype(np.float32),
        d["moving_var"].astype(np.float32))
    full = np.empty((B, N_GROUPS, D_OUT), dtype=np.float32)
    for c, (gs, real) in enumerate(metas):
        m = in_maps[c]
        xg = m["xg"].astype(np.float32).reshape(128, NG, KT, B)
        wt = m["wt"].astype(np.float32).reshape(128, NG, KT, D_OUT)
        cons = m["cons"]
        o = np.empty((NG, 2, 128, B), np.float32)
        for g in range(NG):
            for h in range(2):
                ps = np.zeros((128, B), np.float32)
                for blk in range(KT):
                    ps += wt[:, g, blk, h * 128:(h + 1) * 128].T @ xg[:, g, blk, :]
                z = ps + cons[:, 2 * g + h:2 * g + h + 1]
                t = np.where(z >= 0, z, ALPHA * z)
                o[g, h] = t * cons[:, 12 + h:13 + h] + cons[:, 14 + h:15 + h]
        oc = o.transpose(3, 0, 1, 2).reshape(B, NG, D_OUT)
        full[:, gs, :] = oc[:, :len(gs), :]
    err = np.max(np.abs(full - expected)) / (np.max(np.abs(expected)) + 1e-30)
    print(f"host_check max-abs-rel err = {err:.3e}")
    return err


if __name__ == "__main__":
    host_check()
